# revision 12
# baseline (speedup 1.0000x reference)
"""VQ-VAE codebook kernel for 8 Trainium2 NeuronCores (Bass/Tile).

Problem: nn_BaselineVQVAE (vq_codebook). Full inputs in, full outputs out.
Sharding: data-parallel over flattened token dim N = B*h*w = 65536
(B=16 -> 2 batch slices per core), codebook replicated; counts/sums
all-reduced for the EMA update.

Per-core pipeline (8192 tokens each, 64 token-tiles of 128):
  PE   : scores[t,k] = 2*z.c - |c|^2   (argmax == argmin dist)
  DVE  : -max_k scores (tensor_reduce negate)
  ACT  : eq = Sign(scores - max) in {-1,0}   (exact one-hot marker)
  DVE  : max_index(eq) -> argmin index
  PE   : segment sums/counts via eq.T @ [z|1] matmuls (PSUM-accumulated)
  DMA  : e_k = codebook[idx] via dma_gather; PE-transpose to [d,t] and store
  CC   : AllReduce(sums', counts', Sum z, Sum z^2, Sum max) -> EMA outputs
"""

import sys

sys.path.insert(0, "/opt/trn_rl_repo")

import numpy as np
import concourse.bass as bass
import concourse.bacc as bacc
import concourse.mybir as mybir
import concourse.tile as tile
import concourse.bass_isa as bass_isa
from concourse.bass_utils import run_bass_kernel_spmd

F32 = mybir.dt.float32
F32R = mybir.dt.float32r
F16 = mybir.dt.float16
BF16 = mybir.dt.bfloat16
U16 = mybir.dt.uint16
I16 = mybir.dt.int16
I32 = mybir.dt.int32
AF = mybir.ActivationFunctionType
OP = mybir.AluOpType
AX = mybir.AxisListType

NCORES = 8
B, D, H, W = 16, 64, 64, 64
K = 1024
BPC = B // NCORES          # batch slices per core = 2
TPB = H * W                # tokens per batch slice = 4096
NLOC = BPC * TPB           # tokens per core = 8192
NTILE = NLOC // 128        # 64 token tiles
TPB_TILES = TPB // 128     # 32 tiles per batch slice
NTOT = B * H * W           # 65536
BETA = 0.25
DECAY = 0.99
EPS = 1e-5

_CACHE = {}


def build():
    nc = bacc.Bacc("TRN2", target_bir_lowering=False, debug=False,
                   num_devices=NCORES)

    z_in = nc.dram_tensor("z", [BPC, D, H, W], F32, kind="ExternalInput").ap()
    cb_in = nc.dram_tensor("cb", [K, D], F32, kind="ExternalInput").ap()
    emac_in = nc.dram_tensor("emac", [K], F32, kind="ExternalInput").ap()
    emas_in = nc.dram_tensor("emas", [K, D], F32, kind="ExternalInput").ap()

    e_out = nc.dram_tensor("e_out", [BPC, D, H * W], F32, kind="ExternalOutput").ap()
    idx_out = nc.dram_tensor("idx_out", [BPC * H * W], I32, kind="ExternalOutput").ap()
    l_out = nc.dram_tensor("l_out", [1, 1], F32, kind="ExternalOutput").ap()
    cb_out = nc.dram_tensor("cb_out", [K, D], F32, kind="ExternalOutput").ap()
    cnt_out = nc.dram_tensor("cnt_out", [K], F32, kind="ExternalOutput").ap()
    sum_out = nc.dram_tensor("sum_out", [K, D], F32, kind="ExternalOutput").ap()

    dbg_pre = nc.dram_tensor("dbg_pre", [128, 528], F32, kind="ExternalOutput").ap()
    dbg_post = nc.dram_tensor("dbg_post", [128, 528], F32, kind="ExternalOutput").ap()


    with tile.TileContext(nc) as tc:
        with (
            tc.tile_pool(name="dram", bufs=1, space="DRAM") as dramp,
            tc.tile_pool(name="big", bufs=1) as bigp,
            tc.tile_pool(name="eqp", bufs=3) as eqp,
            tc.tile_pool(name="idxp", bufs=3) as idxp,
            tc.tile_pool(name="smal", bufs=2) as smal,
            tc.tile_pool(name="psc", bufs=2, space="PSUM") as pscp,
            tc.tile_pool(name="pss", bufs=1, space="PSUM") as pssp,
            tc.tile_pool(name="ptr", bufs=2, space="PSUM") as ptrp,
        ):
            # ---------- persistent tiles ----------
            ar_in = dramp.tile([128, 528], F32, tag="arin")
            ar_out = dramp.tile([128, 528], F32, tag="arout", addr_space="Shared")
            z_sb = bigp.tile([65, NLOC], F32, tag="z")        # [z rows 0-63; ones row 64]
            z_bf = bigp.tile([64, NLOC], F16, tag="zbf")
            z_tok = bigp.tile([128, NTILE, 80], F16, tag="ztok")
            cb_sb = bigp.tile([128, 8, 64], F32, tag="cb")
            cb_main = bigp.tile([65, K], F32, tag="cbm")      # [2c^T; -|c|^2]
            id128 = bigp.tile([128, 128], F32, tag="id")
            zeros8 = bigp.tile([128, 8], F16, tag="z8")
            mneg = bigp.tile([128, NTILE], F32, tag="mneg")
            idx32 = bigp.tile([128, NTILE], I32, tag="idx32")
            ar_sb = bigp.tile([128, 528], F32, tag="ar")
            zc_p = bigp.tile([64, 8], F32, tag="zcp")         # per-chunk Sum z accums
            c2 = bigp.tile([128, 8], F32, tag="c2")

            nc.vector.memset(zeros8[:], 0.0)
            nc.vector.memset(z_sb[64:65, :], 1.0)
            nc.vector.memset(z_tok[:, :, 64:65], 1.0)
            nc.vector.memset(ar_sb[:], 0.0)

            # identity for PE transposes
            ones128 = smal.tile([128, 128], F32, tag="ones")
            nc.vector.memset(ones128[:], 1.0)
            nc.gpsimd.affine_select(
                id128[:], ones128[:], pattern=[[1, 128]],
                compare_op=OP.is_equal, fill=0.0, base=0, channel_multiplier=-1,
            )

            # ---------- load codebook + build cb_main = [2c^T ; -|c|^2] ----------
            nc.sync.dma_start(cb_sb[:], cb_in.rearrange("(c p) d -> p c d", p=128))
            for c in range(8):
                pt = ptrp.tile([64, 128], F32, tag="ptr")
                nc.tensor.transpose(pt[:], cb_sb[:, c, :], id128[:])
                nc.scalar.activation(cb_main[0:64, 128 * c:128 * (c + 1)], pt[:],
                                     AF.Copy, scale=2.0)
                sq = smal.tile([128, 64], BF16, tag="sqscr")
                nc.scalar.activation(sq[:], cb_sb[:, c, :], AF.Square,
                                     accum_out=c2[:, c:c + 1])
            ptc2 = ptrp.tile([64, 128], F32, tag="ptr")
            nc.tensor.transpose(ptc2[0:8, :], c2[:], id128[:])
            c2t = smal.tile([8, 128], F32, tag="c2t")
            nc.vector.tensor_scalar_mul(c2t[:], ptc2[0:8, :], -1.0)
            nc.sync.dma_start(
                cb_main[64:65, :].rearrange("x (c p) -> x c p", p=128), c2t[:])

            # ---------- load z, build ones row, z_bf (+ Sum z accum), z_tok ----------
            for b in range(BPC):
                for ch in range(8):
                    sl = slice(b * TPB + ch * 512, b * TPB + (ch + 1) * 512)
                    nc.sync.dma_start(
                        z_sb[0:64, sl],
                        z_in[b].rearrange("d h w -> d (h w)")[:, ch * 512:(ch + 1) * 512])
            for ch in range(8):
                sl = slice(ch * 1024, (ch + 1) * 1024)
                nc.scalar.activation(z_bf[:, sl], z_sb[0:64, sl], AF.Copy)
            # correction must sum the ROUNDED z (what the sums matmuls see)
            zcs = smal.tile([64, 1024], F16, tag="zcscr")
            for ch in range(8):
                sl = slice(ch * 1024, (ch + 1) * 1024)
                nc.scalar.activation(zcs[:], z_bf[:, sl], AF.Copy,
                                     accum_out=zc_p[:, ch:ch + 1])
            for i in range(NTILE):
                nc.sync.dma_start_transpose(
                    z_tok[:, i, 0:64], z_bf[:, 128 * i:128 * (i + 1)])

            # ---------- main loop over 64 token tiles ----------
            psA = pssp.tile([128, 260], F32, tag="psA")
            psB = pssp.tile([128, 260], F32, tag="psB")
            for i in range(NTILE):
                psc = pscp.tile([128, K], F32, tag="sc")
                for h in range(2):
                    nc.tensor.matmul(
                        psc[:, 512 * h:512 * (h + 1)],
                        z_sb[:, 128 * i:128 * (i + 1)],
                        cb_main[:, 512 * h:512 * (h + 1)],
                        start=True, stop=True)
                nc.vector.tensor_reduce(
                    mneg[:, i:i + 1], psc[:], axis=AX.X, op=OP.max, negate=True)
                eq = eqp.tile([128, K], F16, tag="eq")
                nc.scalar.activation(eq[:], psc[:], AF.Sign, bias=mneg[:, i:i + 1])
                idx8 = idxp.tile([128, 8], U16, tag="idx8")
                nc.vector.max_index(idx8[:], zeros8[:], eq[:])
                nc.vector.tensor_copy(idx32[:, i:i + 1], idx8[:, 0:1])
                for c in range(8):
                    ps, off = (psA, 65 * c) if c < 4 else (psB, 65 * (c - 4))
                    nc.tensor.matmul(
                        ps[:, off:off + 65],
                        eq[:, 128 * c:128 * (c + 1)],
                        z_tok[:, i, 0:65],
                        start=(i == 0 and c in (0, 4)),
                        stop=(i == NTILE - 1 and c in (3, 7)))

                # e_k: indirect-gather codebook rows, transpose to [d, t], store
                eg = idxp.tile([128, 64], F32, tag="eg")
                nc.gpsimd.indirect_dma_start(
                    out=eg[:], out_offset=None, in_=cb_in,
                    in_offset=bass.IndirectOffsetOnAxis(ap=idx32[:, i:i + 1], axis=0))
                pt = ptrp.tile([64, 128], F32, tag="ptr")
                nc.tensor.transpose(pt[:], eg[:], id128[:])
                et = eqp.tile([64, 128], F32, tag="et")
                nc.scalar.activation(et[:], pt[:], AF.Copy)
                b, j = i // TPB_TILES, i % TPB_TILES
                nc.sync.dma_start(e_out[b][:, 128 * j:128 * (j + 1)], et[:])

            # indices output (int32), addr = 128*i + p
            nc.sync.dma_start(idx_out.rearrange("(i p) -> p i", p=128), idx32[:])

            # ---------- local stats -> ar buffer ----------
            # Sum z^2 (rows 0-63 of col 520); reuse z_bf as scratch output
            zsq = smal.tile([64, 1], F32, tag="zsq")
            nc.scalar.activation(z_bf[:, :], z_sb[0:64, :], AF.Square,
                                 accum_out=zsq[:])
            nc.vector.tensor_copy(ar_sb[0:64, 520:521], zsq[:])
            # Sum of -max over tokens (col 521)
            nc.vector.tensor_reduce(ar_sb[:, 521:522], mneg[:], axis=AX.X, op=OP.add)
            # Sum z correction vector (col 522, rows 0-63; row 64 = token count)
            corr63 = smal.tile([64, 1], F32, tag="c63")
            nc.vector.tensor_reduce(corr63[:], zc_p[:], axis=AX.X, op=OP.add)
            nc.vector.tensor_copy(ar_sb[0:64, 522:523], corr63[:])
            nc.vector.memset(ar_sb[64:65, 522:523], float(NLOC))
            # sums' chunks (cols 0-519)
            nc.scalar.activation(ar_sb[:, 0:260], psA[:], AF.Copy)
            nc.scalar.activation(ar_sb[:, 260:520], psB[:], AF.Copy)

            # ---------- all-reduce ----------
            nc.sync.dma_start(dbg_pre, ar_sb[:])
            nc.sync.dma_start(ar_in[:], ar_sb[:])
            nc.gpsimd.collective_compute(
                "AllReduce", OP.add,
                replica_groups=[list(range(NCORES))],
                ins=[ar_in[:].opt()], outs=[ar_out[:].opt()])

            arg = bigp.tile([128, 528], F32, tag="arg")
            nc.sync.dma_start(arg[:], ar_out[:])
            nc.sync.dma_start(dbg_post, arg[:])

            # ---------- EMA + loss ----------
            # corr broadcast row: [65,1] -> [1,65] -> [1,520] -> [128,520]
            c_row65 = smal.tile([1, 65], F32, tag="cr65")
            nc.sync.dma_start(c_row65[0:1, :], arg[0:65, 522:523])
            c_row520 = smal.tile([1, 520], F32, tag="cr520")
            for c in range(8):
                nc.sync.dma_start(c_row520[0:1, 65 * c:65 * (c + 1)], c_row65[0:1, :])
            c_bcast = bigp.tile([128, 520], F32, tag="cbc")
            nc.gpsimd.partition_broadcast(c_bcast[:], c_row520[0:1, :])
            sums_g = bigp.tile([128, 8, 65], F32, tag="sg")
            nc.vector.tensor_add(
                sums_g[:].rearrange("p c f -> p (c f)"), arg[:, 0:520], c_bcast[:])

            emac_sb = smal.tile([128, 8], F32, tag="emac")
            nc.sync.dma_start(emac_sb[:], emac_in.rearrange("(c p) -> p c", p=128))
            emas_sb = bigp.tile([128, 8, 64], F32, tag="emas")
            nc.sync.dma_start(emas_sb[:], emas_in.rearrange("(c p) d -> p c d", p=128))

            cnt01 = smal.tile([128, 8], F32, tag="cnt01")
            nc.vector.tensor_scalar_mul(cnt01[:], sums_g[:, :, 64], 1.0 - DECAY)
            ncnt = smal.tile([128, 8], F32, tag="ncnt")
            nc.vector.scalar_tensor_tensor(
                ncnt[:], emac_sb[:], DECAY, cnt01[:], op0=OP.mult, op1=OP.add)

            s01 = bigp.tile([128, 8, 64], F32, tag="s01")
            nc.vector.tensor_scalar_mul(s01[:], sums_g[:, :, 0:64], 1.0 - DECAY)
            nsum = bigp.tile([128, 8, 64], F32, tag="nsum")
            nc.vector.scalar_tensor_tensor(
                nsum[:], emas_sb[:], DECAY, s01[:], op0=OP.mult, op1=OP.add)

            nrs = smal.tile([128, 1], F32, tag="nrs")
            nc.vector.tensor_reduce(nrs[:], ncnt[:], axis=AX.X, op=OP.add)
            n_all = smal.tile([128, 1], F32, tag="nall")
            nc.gpsimd.partition_all_reduce(n_all[:], nrs[:], channels=128,
                                           reduce_op=bass_isa.ReduceOp.add)
            denom = smal.tile([128, 1], F32, tag="den")
            nc.vector.tensor_scalar_add(denom[:], n_all[:], float(K) * EPS)
            rden = smal.tile([128, 1], F32, tag="rden")
            nc.vector.reciprocal(rden[:], denom[:])
            t1 = smal.tile([128, 8], F32, tag="t1")
            nc.vector.tensor_scalar_add(t1[:], ncnt[:], EPS)
            t2 = smal.tile([128, 8], F32, tag="t2")
            nc.vector.tensor_scalar(t2[:], t1[:], rden[:, 0:1], None, op0=OP.mult)
            cs = smal.tile([128, 8], F32, tag="cs")
            nc.vector.tensor_scalar(cs[:], t2[:], n_all[:, 0:1], None, op0=OP.mult)
            ics = smal.tile([128, 8], F32, tag="ics")
            nc.vector.reciprocal(ics[:], cs[:])
            ncb = bigp.tile([128, 8, 64], F32, tag="ncb")
            for c in range(8):
                nc.vector.tensor_scalar(
                    ncb[:, c, :], nsum[:, c, :], ics[:, c:c + 1], None, op0=OP.mult)

            # loss: L = (1+beta)/(N*d) * (Sum z^2 + Sum(-max))
            zs_all = smal.tile([64, 1], F32, tag="zsall")
            nc.gpsimd.partition_all_reduce(zs_all[:], arg[0:64, 520:521], channels=64,
                                           reduce_op=bass_isa.ReduceOp.add)
            mn_all = smal.tile([128, 1], F32, tag="mnall")
            nc.gpsimd.partition_all_reduce(mn_all[:], arg[:, 521:522], channels=128,
                                           reduce_op=bass_isa.ReduceOp.add)
            lsum = smal.tile([1, 1], F32, tag="lsum")
            nc.vector.tensor_add(lsum[:], zs_all[0:1, :], mn_all[0:1, :])
            lfin = smal.tile([1, 1], F32, tag="lfin")
            nc.vector.tensor_scalar_mul(lfin[:], lsum[:], (1.0 + BETA) / (NTOT * D))

            # ---------- final DMAs ----------
            nc.sync.dma_start(l_out, lfin[:])
            nc.sync.dma_start(cnt_out.rearrange("(c p) -> p c", p=128), ncnt[:])
            nc.sync.dma_start(sum_out.rearrange("(c p) d -> p c d", p=128), nsum[:])
            nc.sync.dma_start(cb_out.rearrange("(c p) d -> p c d", p=128), ncb[:])

    nc.compile()
    return nc


def kernel(z_e, codebook, ema_count, ema_sum):
    if "nc" not in _CACHE:
        _CACHE["nc"] = build()
    nc = _CACHE["nc"]

    z_e = np.ascontiguousarray(z_e, dtype=np.float32)
    codebook = np.ascontiguousarray(codebook, dtype=np.float32)
    ema_count = np.ascontiguousarray(ema_count, dtype=np.float32)
    ema_sum = np.ascontiguousarray(ema_sum, dtype=np.float32)

    in_maps = []
    for c in range(NCORES):
        in_maps.append({
            "z": z_e[c * BPC:(c + 1) * BPC],
            "cb": codebook,
            "emac": ema_count,
            "emas": ema_sum,
        })
    res = run_bass_kernel_spmd(nc, in_maps, core_ids=list(range(NCORES)))
    rs = res.results

    e_k_ste = np.concatenate(
        [rs[c]["e_out"].reshape(BPC, D, H, W) for c in range(NCORES)], axis=0)
    indices = np.concatenate(
        [rs[c]["idx_out"].reshape(BPC, H, W) for c in range(NCORES)], axis=0)
    l_commit = np.float32(rs[0]["l_out"][0, 0])
    new_codebook = rs[0]["cb_out"]
    new_count = rs[0]["cnt_out"]
    new_sum = rs[0]["sum_out"]
    return (e_k_ste, indices, l_commit, new_codebook, new_count, new_sum)


# revision 14
# speedup vs baseline: 1.2477x; 1.2477x over previous
"""VQ-VAE codebook kernel for 8 Trainium2 NeuronCores (Bass/Tile).

Problem: nn_BaselineVQVAE (vq_codebook). Full inputs in, full outputs out.
Sharding: data-parallel over flattened token dim N = B*h*w = 65536
(B=16 -> 2 batch slices per core), codebook replicated; counts/sums
all-reduced for the EMA update.

Per-core pipeline (8192 tokens each, 64 token-tiles of 128):
  PE   : scores[t,k] = 2*z.c - |c|^2 via f32r main matmul + f16 residual
         correction matmul (error ~ 1e-5, ~4x faster than fp32 matmul)
  DVE  : -max_k scores (tensor_reduce negate)
  ACT  : eq = Sign(scores - max) in {-1,0}   (exact one-hot marker)
  DVE  : max_index(eq) -> argmin index
  PE   : segment sums/counts via eq.T @ [z|1] matmuls (PSUM-accumulated)
  DMA  : e_k = codebook[idx] via indirect DMA; PE-transpose to [d,t], store
  CC   : AllReduce(sums', counts', Sum z, Sum z^2, Sum max) -> EMA outputs
"""

import sys

sys.path.insert(0, "/opt/trn_rl_repo")

import numpy as np
import concourse.bass as bass
import concourse.bacc as bacc
import concourse.mybir as mybir
import concourse.tile as tile
import concourse.bass_isa as bass_isa
from concourse.bass_utils import run_bass_kernel_spmd

F32 = mybir.dt.float32
F32R = mybir.dt.float32r
F16 = mybir.dt.float16
BF16 = mybir.dt.bfloat16
U16 = mybir.dt.uint16
I16 = mybir.dt.int16
I32 = mybir.dt.int32
AF = mybir.ActivationFunctionType
OP = mybir.AluOpType
AX = mybir.AxisListType

NCORES = 8
B, D, H, W = 16, 64, 64, 64
K = 1024
BPC = B // NCORES          # batch slices per core = 2
TPB = H * W                # tokens per batch slice = 4096
NLOC = BPC * TPB           # tokens per core = 8192
NTILE = NLOC // 128        # 64 token tiles
TPB_TILES = TPB // 128     # 32 tiles per batch slice
NTOT = B * H * W           # 65536
BETA = 0.25
DECAY = 0.99
EPS = 1e-5

_CACHE = {}


def build():
    nc = bacc.Bacc("TRN2", target_bir_lowering=False, debug=False,
                   num_devices=NCORES)

    z_in = nc.dram_tensor("z", [BPC, D, H, W], F32, kind="ExternalInput").ap()
    cb_in = nc.dram_tensor("cb", [K, D], F32, kind="ExternalInput").ap()
    emac_in = nc.dram_tensor("emac", [K], F32, kind="ExternalInput").ap()
    emas_in = nc.dram_tensor("emas", [K, D], F32, kind="ExternalInput").ap()

    e_out = nc.dram_tensor("e_out", [BPC, D, H * W], F32, kind="ExternalOutput").ap()
    idx_out = nc.dram_tensor("idx_out", [BPC * H * W], I32, kind="ExternalOutput").ap()
    l_out = nc.dram_tensor("l_out", [1, 1], F32, kind="ExternalOutput").ap()
    cb_out = nc.dram_tensor("cb_out", [K, D], F32, kind="ExternalOutput").ap()
    cnt_out = nc.dram_tensor("cnt_out", [K], F32, kind="ExternalOutput").ap()
    sum_out = nc.dram_tensor("sum_out", [K, D], F32, kind="ExternalOutput").ap()

    with tile.TileContext(nc) as tc:
        with (
            tc.tile_pool(name="dram", bufs=1, space="DRAM") as dramp,
            tc.tile_pool(name="big", bufs=1) as bigp,
            tc.tile_pool(name="eqp", bufs=3) as eqp,
            tc.tile_pool(name="idxp", bufs=3) as idxp,
            tc.tile_pool(name="etp", bufs=2) as etp,
            tc.tile_pool(name="smal", bufs=2) as smal,
            tc.tile_pool(name="psc", bufs=2, space="PSUM") as pscp,
            tc.tile_pool(name="pss", bufs=1, space="PSUM") as pssp,
            tc.tile_pool(name="ptr", bufs=2, space="PSUM") as ptrp,
        ):
            # ---------- persistent tiles ----------
            ar_in = dramp.tile([128, 528], F32, tag="arin")
            ar_out = dramp.tile([128, 528], F32, tag="arout", addr_space="Shared")
            z_sb = bigp.tile([67, NLOC], F32, tag="z")
            z_mr = bigp.tile([67, NLOC], F32R, tag="zmr")     # [z_r; 1; 1; 1]
            z_cr = bigp.tile([128, NLOC], F16, tag="zcr")     # [z_d; z_r]
            z_bf = bigp.tile([64, NLOC], F16, tag="zbf")
            z_tok = bigp.tile([128, NTILE, 80], F16, tag="ztok")
            cb_sb = bigp.tile([128, 8, 64], F32, tag="cb")
            cb_main = bigp.tile([64, K], F32, tag="cbm")      # 2c^T full fp32
            cb_mr = bigp.tile([67, K], F32R, tag="cbmr")      # [2c_r; -c2hi; -mid; -lo]
            cb_cr = bigp.tile([128, K], F16, tag="cbcr")      # [2c_r(f16); 2c_d]
            id128 = bigp.tile([128, 128], F32, tag="id")
            zeros8 = bigp.tile([128, 8], F16, tag="z8")
            mneg = bigp.tile([128, NTILE], F32, tag="mneg")
            idx32 = bigp.tile([128, NTILE], I32, tag="idx32")
            ar_sb = bigp.tile([128, 528], F32, tag="ar")
            zc_p = bigp.tile([64, 8], F32, tag="zcp")
            c2 = bigp.tile([128, 8], F32, tag="c2")

            nc.vector.memset(zeros8[:], 0.0)
            nc.vector.memset(z_sb[64:67, :], 1.0)
            nc.vector.memset(z_tok[:, :, 64:65], 1.0)
            nc.vector.memset(ar_sb[:], 0.0)

            # identity for PE transposes
            ones128 = smal.tile([128, 128], F32, tag="ones")
            nc.vector.memset(ones128[:], 1.0)
            nc.gpsimd.affine_select(
                id128[:], ones128[:], pattern=[[1, 128]],
                compare_op=OP.is_equal, fill=0.0, base=0, channel_multiplier=-1,
            )

            # ---------- codebook prep ----------
            nc.sync.dma_start(cb_sb[:], cb_in.rearrange("(c p) d -> p c d", p=128))
            for c in range(8):
                pt = ptrp.tile([64, 128], F32, tag="ptr")
                nc.tensor.transpose(pt[:], cb_sb[:, c, :], id128[:])
                nc.scalar.activation(cb_main[:, 128 * c:128 * (c + 1)], pt[:],
                                     AF.Copy, scale=2.0)
                sq = smal.tile([128, 64], BF16, tag="sqscr")
                nc.scalar.activation(sq[:], cb_sb[:, c, :], AF.Square,
                                     accum_out=c2[:, c:c + 1])
            # rows 0-63: 2c_r (f32r) + residual 2c_d (f16) + f16 copy of 2c_r
            nc.vector.tensor_copy(cb_mr[0:64, :], cb_main[:])
            nc.vector.tensor_tensor(cb_cr[64:128, :], cb_main[:],
                                    cb_mr[0:64, :].bitcast(F32), op=OP.subtract)
            nc.scalar.activation(cb_cr[0:64, :], cb_mr[0:64, :].bitcast(F32), AF.Copy)
            # |c|^2 split into three f32r terms: c2 = hi + mid + lo
            c2hi = smal.tile([128, 8], F32R, tag="c2hi")
            nc.vector.tensor_copy(c2hi[:], c2[:])
            r1 = smal.tile([128, 8], F32, tag="r1")
            nc.vector.tensor_tensor(r1[:], c2[:], c2hi[:].bitcast(F32),
                                    op=OP.subtract)
            c2mid = smal.tile([128, 8], F32R, tag="c2mid")
            nc.vector.tensor_copy(c2mid[:], r1[:])
            c2lo = smal.tile([128, 8], F32, tag="c2lo")
            nc.vector.tensor_tensor(c2lo[:], r1[:], c2mid[:].bitcast(F32),
                                    op=OP.subtract)
            n3 = smal.tile([128, 24], F32, tag="n3")
            nc.vector.tensor_scalar_mul(n3[:, 0:8], c2hi[:].bitcast(F32), -1.0)
            nc.vector.tensor_scalar_mul(n3[:, 8:16], c2mid[:].bitcast(F32), -1.0)
            nc.vector.tensor_scalar_mul(n3[:, 16:24], c2lo[:], -1.0)
            pt24 = ptrp.tile([64, 128], F32, tag="ptr")
            nc.tensor.transpose(pt24[0:24, :], n3[:], id128[:])
            st24 = smal.tile([24, 128], F32, tag="st24")
            nc.vector.tensor_copy(st24[:], pt24[0:24, :])
            rows3 = smal.tile([3, K], F32, tag="rows3")
            for j in range(3):
                nc.sync.dma_start(
                    rows3[j:j + 1, :].rearrange("x (c p) -> x c p", p=128),
                    st24[8 * j:8 * (j + 1), :])
            nc.vector.tensor_copy(cb_mr[64:67, :], rows3[:])

            # ---------- z prep (chunked for pipelining) ----------
            for b in range(BPC):
                for ch in range(8):
                    sl = slice(b * TPB + ch * 512, b * TPB + (ch + 1) * 512)
                    nc.sync.dma_start(
                        z_sb[0:64, sl],
                        z_in[b].rearrange("d h w -> d (h w)")[:, ch * 512:(ch + 1) * 512])
            for ch in range(8):
                sl = slice(ch * 1024, (ch + 1) * 1024)
                nc.vector.tensor_copy(z_mr[:, sl], z_sb[:, sl])
                nc.vector.tensor_tensor(z_cr[0:64, sl], z_sb[0:64, sl],
                                        z_mr[0:64, sl].bitcast(F32), op=OP.subtract)
                nc.scalar.activation(z_cr[64:128, sl], z_mr[0:64, sl].bitcast(F32),
                                     AF.Copy)
                nc.scalar.activation(z_bf[:, sl], z_sb[0:64, sl], AF.Copy)
            # correction must sum the ROUNDED z (what the sums matmuls see)
            zcs = smal.tile([64, 1024], F16, tag="zcscr")
            for ch in range(8):
                sl = slice(ch * 1024, (ch + 1) * 1024)
                nc.scalar.activation(zcs[:], z_bf[:, sl], AF.Copy,
                                     accum_out=zc_p[:, ch:ch + 1])
            for i in range(NTILE):
                nc.sync.dma_start_transpose(
                    z_tok[:, i, 0:64], z_bf[:, 128 * i:128 * (i + 1)])

            # ---------- main loop over 64 token tiles ----------
            psA = pssp.tile([128, 260], F32, tag="psA")
            psB = pssp.tile([128, 260], F32, tag="psB")
            et = None
            for i in range(NTILE):
                psc = pscp.tile([128, K], F32, tag="sc")
                for h in range(2):
                    nc.tensor.matmul(
                        psc[:, 512 * h:512 * (h + 1)],
                        z_mr[:, 128 * i:128 * (i + 1)],
                        cb_mr[:, 512 * h:512 * (h + 1)],
                        start=True, stop=False)
                    nc.tensor.matmul(
                        psc[:, 512 * h:512 * (h + 1)],
                        z_cr[:, 128 * i:128 * (i + 1)],
                        cb_cr[:, 512 * h:512 * (h + 1)],
                        start=False, stop=True)
                nc.vector.tensor_reduce(
                    mneg[:, i:i + 1], psc[:], axis=AX.X, op=OP.max, negate=True)
                eq = eqp.tile([128, K], F16, tag="eq")
                nc.scalar.activation(eq[:], psc[:], AF.Sign, bias=mneg[:, i:i + 1])
                idx8 = idxp.tile([128, 8], U16, tag="idx8")
                nc.vector.max_index(idx8[:], zeros8[:], eq[:])
                nc.vector.tensor_copy(idx32[:, i:i + 1], idx8[:, 0:1])
                for c in range(8):
                    ps, off = (psA, 65 * c) if c < 4 else (psB, 65 * (c - 4))
                    nc.tensor.matmul(
                        ps[:, off:off + 65],
                        eq[:, 128 * c:128 * (c + 1)],
                        z_tok[:, i, 0:65],
                        start=(i == 0 and c in (0, 4)),
                        stop=(i == NTILE - 1 and c in (3, 7)))

                # e_k: indirect-gather codebook rows, transpose to [d, t], store
                eg = idxp.tile([128, 64], F32, tag="eg")
                nc.gpsimd.indirect_dma_start(
                    out=eg[:], out_offset=None, in_=cb_in,
                    in_offset=bass.IndirectOffsetOnAxis(ap=idx32[:, i:i + 1], axis=0))
                pt = ptrp.tile([64, 128], F32, tag="ptr")
                nc.tensor.transpose(pt[:], eg[:], id128[:])
                if i % 4 == 0:
                    et = etp.tile([64, 512], F32, tag="et")
                nc.scalar.activation(et[:, 128 * (i % 4):128 * (i % 4 + 1)], pt[:],
                                     AF.Copy)
                if i % 4 == 3:
                    b, j = i // TPB_TILES, (i % TPB_TILES) // 4
                    nc.sync.dma_start(e_out[b][:, 512 * j:512 * (j + 1)], et[:])

            # indices output (int32), addr = 128*i + p
            nc.sync.dma_start(idx_out.rearrange("(i p) -> p i", p=128), idx32[:])

            # ---------- local stats -> ar buffer ----------
            zsq = smal.tile([64, 1], F32, tag="zsq")
            nc.scalar.activation(z_bf[:, :], z_sb[0:64, :], AF.Square,
                                 accum_out=zsq[:])
            nc.vector.tensor_copy(ar_sb[0:64, 520:521], zsq[:])
            nc.vector.tensor_reduce(ar_sb[:, 521:522], mneg[:], axis=AX.X, op=OP.add)
            corr63 = smal.tile([64, 1], F32, tag="c63")
            nc.vector.tensor_reduce(corr63[:], zc_p[:], axis=AX.X, op=OP.add)
            nc.vector.tensor_copy(ar_sb[0:64, 522:523], corr63[:])
            nc.vector.memset(ar_sb[64:65, 522:523], float(NLOC))
            nc.scalar.activation(ar_sb[:, 0:260], psA[:], AF.Copy)
            nc.scalar.activation(ar_sb[:, 260:520], psB[:], AF.Copy)

            # ---------- all-reduce ----------
            nc.sync.dma_start(ar_in[:], ar_sb[:])
            nc.gpsimd.collective_compute(
                "AllReduce", OP.add,
                replica_groups=[list(range(NCORES))],
                ins=[ar_in[:].opt()], outs=[ar_out[:].opt()])

            arg = bigp.tile([128, 528], F32, tag="arg")
            nc.sync.dma_start(arg[:], ar_out[:])

            # ---------- EMA + loss ----------
            c_row65 = smal.tile([1, 65], F32, tag="cr65")
            nc.sync.dma_start(c_row65[0:1, :], arg[0:65, 522:523])
            c_row520 = smal.tile([1, 520], F32, tag="cr520")
            for c in range(8):
                nc.sync.dma_start(c_row520[0:1, 65 * c:65 * (c + 1)], c_row65[0:1, :])
            c_bcast = bigp.tile([128, 520], F32, tag="cbc")
            nc.gpsimd.partition_broadcast(c_bcast[:], c_row520[0:1, :])
            sums_g = bigp.tile([128, 8, 65], F32, tag="sg")
            nc.vector.tensor_add(
                sums_g[:].rearrange("p c f -> p (c f)"), arg[:, 0:520], c_bcast[:])

            emac_sb = smal.tile([128, 8], F32, tag="emac")
            nc.sync.dma_start(emac_sb[:], emac_in.rearrange("(c p) -> p c", p=128))
            emas_sb = bigp.tile([128, 8, 64], F32, tag="emas")
            nc.sync.dma_start(emas_sb[:], emas_in.rearrange("(c p) d -> p c d", p=128))

            cnt01 = smal.tile([128, 8], F32, tag="cnt01")
            nc.vector.tensor_scalar_mul(cnt01[:], sums_g[:, :, 64], 1.0 - DECAY)
            ncnt = smal.tile([128, 8], F32, tag="ncnt")
            nc.vector.scalar_tensor_tensor(
                ncnt[:], emac_sb[:], DECAY, cnt01[:], op0=OP.mult, op1=OP.add)

            s01 = bigp.tile([128, 8, 64], F32, tag="s01")
            nc.vector.tensor_scalar_mul(s01[:], sums_g[:, :, 0:64], 1.0 - DECAY)
            nsum = bigp.tile([128, 8, 64], F32, tag="nsum")
            nc.vector.scalar_tensor_tensor(
                nsum[:], emas_sb[:], DECAY, s01[:], op0=OP.mult, op1=OP.add)

            nrs = smal.tile([128, 1], F32, tag="nrs")
            nc.vector.tensor_reduce(nrs[:], ncnt[:], axis=AX.X, op=OP.add)
            n_all = smal.tile([128, 1], F32, tag="nall")
            nc.gpsimd.partition_all_reduce(n_all[:], nrs[:], channels=128,
                                           reduce_op=bass_isa.ReduceOp.add)
            denom = smal.tile([128, 1], F32, tag="den")
            nc.vector.tensor_scalar_add(denom[:], n_all[:], float(K) * EPS)
            rden = smal.tile([128, 1], F32, tag="rden")
            nc.vector.reciprocal(rden[:], denom[:])
            t1 = smal.tile([128, 8], F32, tag="t1")
            nc.vector.tensor_scalar_add(t1[:], ncnt[:], EPS)
            t2 = smal.tile([128, 8], F32, tag="t2")
            nc.vector.tensor_scalar(t2[:], t1[:], rden[:, 0:1], None, op0=OP.mult)
            cs = smal.tile([128, 8], F32, tag="cs")
            nc.vector.tensor_scalar(cs[:], t2[:], n_all[:, 0:1], None, op0=OP.mult)
            ics = smal.tile([128, 8], F32, tag="ics")
            nc.vector.reciprocal(ics[:], cs[:])
            ncb = bigp.tile([128, 8, 64], F32, tag="ncb")
            for c in range(8):
                nc.vector.tensor_scalar(
                    ncb[:, c, :], nsum[:, c, :], ics[:, c:c + 1], None, op0=OP.mult)

            # loss: L = (1+beta)/(N*d) * (Sum z^2 + Sum(-max))
            zs_all = smal.tile([64, 1], F32, tag="zsall")
            nc.gpsimd.partition_all_reduce(zs_all[:], arg[0:64, 520:521], channels=64,
                                           reduce_op=bass_isa.ReduceOp.add)
            mn_all = smal.tile([128, 1], F32, tag="mnall")
            nc.gpsimd.partition_all_reduce(mn_all[:], arg[:, 521:522], channels=128,
                                           reduce_op=bass_isa.ReduceOp.add)
            lsum = smal.tile([1, 1], F32, tag="lsum")
            nc.vector.tensor_add(lsum[:], zs_all[0:1, :], mn_all[0:1, :])
            lfin = smal.tile([1, 1], F32, tag="lfin")
            nc.vector.tensor_scalar_mul(lfin[:], lsum[:], (1.0 + BETA) / (NTOT * D))

            # ---------- final DMAs ----------
            nc.sync.dma_start(l_out, lfin[:])
            nc.sync.dma_start(cnt_out.rearrange("(c p) -> p c", p=128), ncnt[:])
            nc.sync.dma_start(sum_out.rearrange("(c p) d -> p c d", p=128), nsum[:])
            nc.sync.dma_start(cb_out.rearrange("(c p) d -> p c d", p=128), ncb[:])

    nc.compile()
    return nc


def kernel(z_e, codebook, ema_count, ema_sum):
    if "nc" not in _CACHE:
        _CACHE["nc"] = build()
    nc = _CACHE["nc"]

    z_e = np.ascontiguousarray(z_e, dtype=np.float32)
    codebook = np.ascontiguousarray(codebook, dtype=np.float32)
    ema_count = np.ascontiguousarray(ema_count, dtype=np.float32)
    ema_sum = np.ascontiguousarray(ema_sum, dtype=np.float32)

    in_maps = []
    for c in range(NCORES):
        in_maps.append({
            "z": z_e[c * BPC:(c + 1) * BPC],
            "cb": codebook,
            "emac": ema_count,
            "emas": ema_sum,
        })
    res = run_bass_kernel_spmd(nc, in_maps, core_ids=list(range(NCORES)))
    rs = res.results

    e_k_ste = np.concatenate(
        [rs[c]["e_out"].reshape(BPC, D, H, W) for c in range(NCORES)], axis=0)
    indices = np.concatenate(
        [rs[c]["idx_out"].reshape(BPC, H, W) for c in range(NCORES)], axis=0)
    l_commit = np.float32(rs[0]["l_out"][0, 0])
    new_codebook = rs[0]["cb_out"]
    new_count = rs[0]["cnt_out"]
    new_sum = rs[0]["sum_out"]
    return (e_k_ste, indices, l_commit, new_codebook, new_count, new_sum)


# revision 15
# speedup vs baseline: 1.5353x; 1.2305x over previous
"""VQ-VAE codebook kernel for 8 Trainium2 NeuronCores (Bass/Tile).

Problem: nn_BaselineVQVAE (vq_codebook). Full inputs in, full outputs out.
Sharding: data-parallel over flattened token dim N = B*h*w = 65536
(B=16 -> 2 batch slices per core), codebook replicated; counts/sums
all-reduced for the EMA update.

Per-core pipeline (8192 tokens each, 64 token-tiles of 128):
  PE   : scores[t,k] = 2*z.c - |c|^2 via f32r main matmul + f16 residual
         correction matmul (error ~ 1e-5, ~4x faster than fp32 matmul)
  DVE  : -max_k scores (tensor_reduce negate)
  ACT  : eq = Sign(scores - max) in {-1,0}   (exact one-hot marker)
  DVE  : max_index(eq) -> argmin index
  PE   : segment sums/counts via eq.T @ [z|1] matmuls (PSUM-accumulated)
  DMA  : e_k = codebook[idx] via indirect DMA; PE-transpose to [d,t], store
  CC   : AllReduce(sums', counts', Sum z, Sum z^2, Sum max) -> EMA outputs
"""

import sys

sys.path.insert(0, "/opt/trn_rl_repo")

import numpy as np
import concourse.bass as bass
import concourse.bacc as bacc
import concourse.mybir as mybir
import concourse.tile as tile
import concourse.bass_isa as bass_isa
from concourse.bass_utils import run_bass_kernel_spmd

F32 = mybir.dt.float32
F32R = mybir.dt.float32r
F16 = mybir.dt.float16
BF16 = mybir.dt.bfloat16
U16 = mybir.dt.uint16
I16 = mybir.dt.int16
I32 = mybir.dt.int32
AF = mybir.ActivationFunctionType
OP = mybir.AluOpType
AX = mybir.AxisListType

NCORES = 8
B, D, H, W = 16, 64, 64, 64
K = 1024
BPC = B // NCORES          # batch slices per core = 2
TPB = H * W                # tokens per batch slice = 4096
NLOC = BPC * TPB           # tokens per core = 8192
NTILE = NLOC // 128        # 64 token tiles
TPB_TILES = TPB // 128     # 32 tiles per batch slice
NTOT = B * H * W           # 65536
BETA = 0.25
DECAY = 0.99
EPS = 1e-5

_CACHE = {}


def build():
    nc = bacc.Bacc("TRN2", target_bir_lowering=False, debug=False,
                   num_devices=NCORES)

    z_in = nc.dram_tensor("z", [BPC, D, H, W], F32, kind="ExternalInput").ap()
    cb_in = nc.dram_tensor("cb", [K, D], F32, kind="ExternalInput").ap()
    emac_in = nc.dram_tensor("emac", [K], F32, kind="ExternalInput").ap()
    emas_in = nc.dram_tensor("emas", [K, D], F32, kind="ExternalInput").ap()

    e_out = nc.dram_tensor("e_out", [BPC, D, H * W], F32, kind="ExternalOutput").ap()
    idx_out = nc.dram_tensor("idx_out", [BPC * H * W], I32, kind="ExternalOutput").ap()
    l_out = nc.dram_tensor("l_out", [1, 1], F32, kind="ExternalOutput").ap()
    cb_out = nc.dram_tensor("cb_out", [K, D], F32, kind="ExternalOutput").ap()
    cnt_out = nc.dram_tensor("cnt_out", [K], F32, kind="ExternalOutput").ap()
    sum_out = nc.dram_tensor("sum_out", [K, D], F32, kind="ExternalOutput").ap()

    with tile.TileContext(nc) as tc:
        with (
            tc.tile_pool(name="dram", bufs=1, space="DRAM") as dramp,
            tc.tile_pool(name="big", bufs=1) as bigp,
            tc.tile_pool(name="eqp", bufs=3) as eqp,
            tc.tile_pool(name="idxp", bufs=3) as idxp,
            tc.tile_pool(name="etp", bufs=2) as etp,
            tc.tile_pool(name="smal", bufs=2) as smal,
            tc.tile_pool(name="psc", bufs=2, space="PSUM") as pscp,
            tc.tile_pool(name="pss", bufs=1, space="PSUM") as pssp,
            tc.tile_pool(name="ptr", bufs=2, space="PSUM") as ptrp,
        ):
            # ---------- persistent tiles ----------
            ar_in = dramp.tile([128, 528], F32, tag="arin")
            ar_out = dramp.tile([128, 528], F32, tag="arout", addr_space="Shared")
            z_sb = bigp.tile([67, NLOC], F32, tag="z")
            z_mr = bigp.tile([67, NLOC], F32R, tag="zmr")     # [z_r; 1; 1; 1]
            z_cr = bigp.tile([128, NLOC], F16, tag="zcr")     # [z_d; z_r]
            z_bf = bigp.tile([64, NLOC], F16, tag="zbf")
            z_tok = bigp.tile([128, NTILE, 80], F16, tag="ztok")
            cb_sb = bigp.tile([128, 8, 64], F32, tag="cb")
            cb_main = bigp.tile([64, K], F32, tag="cbm")      # 2c^T full fp32
            cb_mr = bigp.tile([67, K], F32R, tag="cbmr")      # [2c_r; -c2hi; -mid; -lo]
            cb_cr = bigp.tile([128, K], F16, tag="cbcr")      # [2c_r(f16); 2c_d]
            id128 = bigp.tile([128, 128], F32, tag="id")
            zeros8 = bigp.tile([128, 8], F16, tag="z8")
            mneg = bigp.tile([128, NTILE], F32, tag="mneg")
            idx32 = bigp.tile([128, NTILE], I32, tag="idx32")
            ar_sb = bigp.tile([128, 528], F32, tag="ar")
            zc_p = bigp.tile([64, 8], F32, tag="zcp")
            c2 = bigp.tile([128, 8], F32, tag="c2")

            nc.vector.memset(zeros8[:], 0.0)
            nc.vector.memset(z_sb[64:67, :], 1.0)
            nc.vector.memset(z_tok[:, :, 64:65], 1.0)
            nc.vector.memset(ar_sb[:], 0.0)

            # identity for PE transposes
            ones128 = smal.tile([128, 128], F32, tag="ones")
            nc.vector.memset(ones128[:], 1.0)
            nc.gpsimd.affine_select(
                id128[:], ones128[:], pattern=[[1, 128]],
                compare_op=OP.is_equal, fill=0.0, base=0, channel_multiplier=-1,
            )

            # ---------- codebook prep ----------
            nc.sync.dma_start(cb_sb[:], cb_in.rearrange("(c p) d -> p c d", p=128))
            for c in range(8):
                pt = ptrp.tile([64, 128], F32, tag="ptr")
                nc.tensor.transpose(pt[:], cb_sb[:, c, :], id128[:])
                nc.scalar.activation(cb_main[:, 128 * c:128 * (c + 1)], pt[:],
                                     AF.Copy, scale=2.0)
                sq = smal.tile([128, 64], BF16, tag="sqscr")
                nc.scalar.activation(sq[:], cb_sb[:, c, :], AF.Square,
                                     accum_out=c2[:, c:c + 1])
            # rows 0-63: 2c_r (f32r) + residual 2c_d (f16) + f16 copy of 2c_r
            nc.vector.tensor_copy(cb_mr[0:64, :], cb_main[:])
            nc.vector.tensor_tensor(cb_cr[64:128, :], cb_main[:],
                                    cb_mr[0:64, :].bitcast(F32), op=OP.subtract)
            nc.scalar.activation(cb_cr[0:64, :], cb_mr[0:64, :].bitcast(F32), AF.Copy)
            # |c|^2 split into three f32r terms: c2 = hi + mid + lo
            c2hi = smal.tile([128, 8], F32R, tag="c2hi")
            nc.vector.tensor_copy(c2hi[:], c2[:])
            r1 = smal.tile([128, 8], F32, tag="r1")
            nc.vector.tensor_tensor(r1[:], c2[:], c2hi[:].bitcast(F32),
                                    op=OP.subtract)
            c2mid = smal.tile([128, 8], F32R, tag="c2mid")
            nc.vector.tensor_copy(c2mid[:], r1[:])
            c2lo = smal.tile([128, 8], F32, tag="c2lo")
            nc.vector.tensor_tensor(c2lo[:], r1[:], c2mid[:].bitcast(F32),
                                    op=OP.subtract)
            n3 = smal.tile([128, 24], F32, tag="n3")
            nc.vector.tensor_scalar_mul(n3[:, 0:8], c2hi[:].bitcast(F32), -1.0)
            nc.vector.tensor_scalar_mul(n3[:, 8:16], c2mid[:].bitcast(F32), -1.0)
            nc.vector.tensor_scalar_mul(n3[:, 16:24], c2lo[:], -1.0)
            pt24 = ptrp.tile([64, 128], F32, tag="ptr")
            nc.tensor.transpose(pt24[0:24, :], n3[:], id128[:])
            st24 = smal.tile([24, 128], F32, tag="st24")
            nc.vector.tensor_copy(st24[:], pt24[0:24, :])
            rows3 = smal.tile([3, K], F32, tag="rows3")
            for j in range(3):
                nc.sync.dma_start(
                    rows3[j:j + 1, :].rearrange("x (c p) -> x c p", p=128),
                    st24[8 * j:8 * (j + 1), :])
            nc.vector.tensor_copy(cb_mr[64:67, :], rows3[:])

            # ---------- z prep (chunked for pipelining) ----------
            for b in range(BPC):
                for ch in range(8):
                    sl = slice(b * TPB + ch * 512, b * TPB + (ch + 1) * 512)
                    nc.sync.dma_start(
                        z_sb[0:64, sl],
                        z_in[b].rearrange("d h w -> d (h w)")[:, ch * 512:(ch + 1) * 512])
            for ch in range(8):
                sl = slice(ch * 1024, (ch + 1) * 1024)
                nc.vector.tensor_copy(z_mr[:, sl], z_sb[:, sl])
                nc.vector.tensor_tensor(z_cr[0:64, sl], z_sb[0:64, sl],
                                        z_mr[0:64, sl].bitcast(F32), op=OP.subtract)
                nc.scalar.activation(z_cr[64:128, sl], z_mr[0:64, sl].bitcast(F32),
                                     AF.Copy)
                nc.scalar.activation(z_bf[:, sl], z_sb[0:64, sl], AF.Copy)
            # correction must sum the ROUNDED z (what the sums matmuls see)
            zcs = smal.tile([64, 1024], F16, tag="zcscr")
            for ch in range(8):
                sl = slice(ch * 1024, (ch + 1) * 1024)
                nc.scalar.activation(zcs[:], z_bf[:, sl], AF.Copy,
                                     accum_out=zc_p[:, ch:ch + 1])
            for b in range(BPC):
                nc.sync.dma_start_transpose(
                    z_tok[:, b * TPB_TILES:(b + 1) * TPB_TILES, 0:64],
                    z_bf[:, b * TPB:(b + 1) * TPB])

            # ---------- main loop over 64 token tiles ----------
            psA = pssp.tile([128, 260], F32, tag="psA")
            psB = pssp.tile([128, 260], F32, tag="psB")
            et = None
            for i in range(NTILE):
                psc = pscp.tile([128, K], F32, tag="sc")
                for h in range(2):
                    nc.tensor.matmul(
                        psc[:, 512 * h:512 * (h + 1)],
                        z_mr[:, 128 * i:128 * (i + 1)],
                        cb_mr[:, 512 * h:512 * (h + 1)],
                        start=True, stop=False)
                    nc.tensor.matmul(
                        psc[:, 512 * h:512 * (h + 1)],
                        z_cr[:, 128 * i:128 * (i + 1)],
                        cb_cr[:, 512 * h:512 * (h + 1)],
                        start=False, stop=True)
                nc.vector.tensor_reduce(
                    mneg[:, i:i + 1], psc[:], axis=AX.X, op=OP.max, negate=True)
                eq = eqp.tile([128, K], F16, tag="eq")
                nc.scalar.activation(eq[:], psc[:], AF.Sign, bias=mneg[:, i:i + 1])
                idx8 = idxp.tile([128, 8], U16, tag="idx8")
                nc.vector.max_index(idx8[:], zeros8[:], eq[:])
                nc.vector.tensor_copy(idx32[:, i:i + 1], idx8[:, 0:1])
                for c in range(8):
                    ps, off = (psA, 65 * c) if c < 4 else (psB, 65 * (c - 4))
                    nc.tensor.matmul(
                        ps[:, off:off + 65],
                        eq[:, 128 * c:128 * (c + 1)],
                        z_tok[:, i, 0:65],
                        start=(i == 0 and c in (0, 4)),
                        stop=(i == NTILE - 1 and c in (3, 7)))

                # e_k: indirect-gather codebook rows, transpose to [d, t], store
                eg = idxp.tile([128, 64], F32, tag="eg")
                nc.gpsimd.indirect_dma_start(
                    out=eg[:], out_offset=None, in_=cb_in,
                    in_offset=bass.IndirectOffsetOnAxis(ap=idx32[:, i:i + 1], axis=0))
                pt = ptrp.tile([64, 128], F32, tag="ptr")
                nc.tensor.transpose(pt[:], eg[:], id128[:])
                if i % 4 == 0:
                    et = etp.tile([64, 512], F32, tag="et")
                nc.scalar.activation(et[:, 128 * (i % 4):128 * (i % 4 + 1)], pt[:],
                                     AF.Copy)
                if i % 4 == 3:
                    b, j = i // TPB_TILES, (i % TPB_TILES) // 4
                    nc.scalar.dma_start(e_out[b][:, 512 * j:512 * (j + 1)], et[:])

            # indices output (int32), addr = 128*i + p
            nc.sync.dma_start(idx_out.rearrange("(i p) -> p i", p=128), idx32[:])

            # ---------- local stats -> ar buffer ----------
            zsq = smal.tile([64, 1], F32, tag="zsq")
            nc.scalar.activation(z_bf[:, :], z_sb[0:64, :], AF.Square,
                                 accum_out=zsq[:])
            nc.vector.tensor_copy(ar_sb[0:64, 520:521], zsq[:])
            nc.vector.tensor_reduce(ar_sb[:, 521:522], mneg[:], axis=AX.X, op=OP.add)
            corr63 = smal.tile([64, 1], F32, tag="c63")
            nc.vector.tensor_reduce(corr63[:], zc_p[:], axis=AX.X, op=OP.add)
            nc.vector.tensor_copy(ar_sb[0:64, 522:523], corr63[:])
            nc.vector.memset(ar_sb[64:65, 522:523], float(NLOC))
            nc.scalar.activation(ar_sb[:, 0:260], psA[:], AF.Copy)
            nc.scalar.activation(ar_sb[:, 260:520], psB[:], AF.Copy)

            # ---------- all-reduce ----------
            nc.sync.dma_start(ar_in[:], ar_sb[:])
            nc.gpsimd.collective_compute(
                "AllReduce", OP.add,
                replica_groups=[list(range(NCORES))],
                ins=[ar_in[:].opt()], outs=[ar_out[:].opt()])

            arg = bigp.tile([128, 528], F32, tag="arg")
            nc.sync.dma_start(arg[:], ar_out[:])

            # ---------- EMA + loss ----------
            c_row65 = smal.tile([1, 65], F32, tag="cr65")
            nc.sync.dma_start(c_row65[0:1, :], arg[0:65, 522:523])
            c_row520 = smal.tile([1, 520], F32, tag="cr520")
            for c in range(8):
                nc.sync.dma_start(c_row520[0:1, 65 * c:65 * (c + 1)], c_row65[0:1, :])
            c_bcast = bigp.tile([128, 520], F32, tag="cbc")
            nc.gpsimd.partition_broadcast(c_bcast[:], c_row520[0:1, :])
            sums_g = bigp.tile([128, 8, 65], F32, tag="sg")
            nc.vector.tensor_add(
                sums_g[:].rearrange("p c f -> p (c f)"), arg[:, 0:520], c_bcast[:])

            emac_sb = smal.tile([128, 8], F32, tag="emac")
            nc.sync.dma_start(emac_sb[:], emac_in.rearrange("(c p) -> p c", p=128))
            emas_sb = bigp.tile([128, 8, 64], F32, tag="emas")
            nc.sync.dma_start(emas_sb[:], emas_in.rearrange("(c p) d -> p c d", p=128))

            cnt01 = smal.tile([128, 8], F32, tag="cnt01")
            nc.vector.tensor_scalar_mul(cnt01[:], sums_g[:, :, 64], 1.0 - DECAY)
            ncnt = smal.tile([128, 8], F32, tag="ncnt")
            nc.vector.scalar_tensor_tensor(
                ncnt[:], emac_sb[:], DECAY, cnt01[:], op0=OP.mult, op1=OP.add)

            s01 = bigp.tile([128, 8, 64], F32, tag="s01")
            nc.vector.tensor_scalar_mul(s01[:], sums_g[:, :, 0:64], 1.0 - DECAY)
            nsum = bigp.tile([128, 8, 64], F32, tag="nsum")
            nc.vector.scalar_tensor_tensor(
                nsum[:], emas_sb[:], DECAY, s01[:], op0=OP.mult, op1=OP.add)

            nrs = smal.tile([128, 1], F32, tag="nrs")
            nc.vector.tensor_reduce(nrs[:], ncnt[:], axis=AX.X, op=OP.add)
            n_all = smal.tile([128, 1], F32, tag="nall")
            nc.gpsimd.partition_all_reduce(n_all[:], nrs[:], channels=128,
                                           reduce_op=bass_isa.ReduceOp.add)
            denom = smal.tile([128, 1], F32, tag="den")
            nc.vector.tensor_scalar_add(denom[:], n_all[:], float(K) * EPS)
            rden = smal.tile([128, 1], F32, tag="rden")
            nc.vector.reciprocal(rden[:], denom[:])
            t1 = smal.tile([128, 8], F32, tag="t1")
            nc.vector.tensor_scalar_add(t1[:], ncnt[:], EPS)
            t2 = smal.tile([128, 8], F32, tag="t2")
            nc.vector.tensor_scalar(t2[:], t1[:], rden[:, 0:1], None, op0=OP.mult)
            cs = smal.tile([128, 8], F32, tag="cs")
            nc.vector.tensor_scalar(cs[:], t2[:], n_all[:, 0:1], None, op0=OP.mult)
            ics = smal.tile([128, 8], F32, tag="ics")
            nc.vector.reciprocal(ics[:], cs[:])
            ncb = bigp.tile([128, 8, 64], F32, tag="ncb")
            for c in range(8):
                nc.vector.tensor_scalar(
                    ncb[:, c, :], nsum[:, c, :], ics[:, c:c + 1], None, op0=OP.mult)

            # loss: L = (1+beta)/(N*d) * (Sum z^2 + Sum(-max))
            zs_all = smal.tile([64, 1], F32, tag="zsall")
            nc.gpsimd.partition_all_reduce(zs_all[:], arg[0:64, 520:521], channels=64,
                                           reduce_op=bass_isa.ReduceOp.add)
            mn_all = smal.tile([128, 1], F32, tag="mnall")
            nc.gpsimd.partition_all_reduce(mn_all[:], arg[:, 521:522], channels=128,
                                           reduce_op=bass_isa.ReduceOp.add)
            lsum = smal.tile([1, 1], F32, tag="lsum")
            nc.vector.tensor_add(lsum[:], zs_all[0:1, :], mn_all[0:1, :])
            lfin = smal.tile([1, 1], F32, tag="lfin")
            nc.vector.tensor_scalar_mul(lfin[:], lsum[:], (1.0 + BETA) / (NTOT * D))

            # ---------- final DMAs ----------
            nc.sync.dma_start(l_out, lfin[:])
            nc.sync.dma_start(cnt_out.rearrange("(c p) -> p c", p=128), ncnt[:])
            nc.sync.dma_start(sum_out.rearrange("(c p) d -> p c d", p=128), nsum[:])
            nc.sync.dma_start(cb_out.rearrange("(c p) d -> p c d", p=128), ncb[:])

    nc.compile()
    return nc


def kernel(z_e, codebook, ema_count, ema_sum):
    if "nc" not in _CACHE:
        _CACHE["nc"] = build()
    nc = _CACHE["nc"]

    z_e = np.ascontiguousarray(z_e, dtype=np.float32)
    codebook = np.ascontiguousarray(codebook, dtype=np.float32)
    ema_count = np.ascontiguousarray(ema_count, dtype=np.float32)
    ema_sum = np.ascontiguousarray(ema_sum, dtype=np.float32)

    in_maps = []
    for c in range(NCORES):
        in_maps.append({
            "z": z_e[c * BPC:(c + 1) * BPC],
            "cb": codebook,
            "emac": ema_count,
            "emas": ema_sum,
        })
    res = run_bass_kernel_spmd(nc, in_maps, core_ids=list(range(NCORES)))
    rs = res.results

    e_k_ste = np.concatenate(
        [rs[c]["e_out"].reshape(BPC, D, H, W) for c in range(NCORES)], axis=0)
    indices = np.concatenate(
        [rs[c]["idx_out"].reshape(BPC, H, W) for c in range(NCORES)], axis=0)
    l_commit = np.float32(rs[0]["l_out"][0, 0])
    new_codebook = rs[0]["cb_out"]
    new_count = rs[0]["cnt_out"]
    new_sum = rs[0]["sum_out"]
    return (e_k_ste, indices, l_commit, new_codebook, new_count, new_sum)
